# revision 30
# baseline (speedup 1.0000x reference)
"""DescripNet (DGCNN EdgeConv x2 + attention pooling) on 8 trn2 cores — v2.

Sharding: core c -> (cloud b = c//2, half = c%2); 2048 dst rows/core.

v2 changes vs baseline:
  - knn score matmuls in fp32r (1 cy/row vs 4 for fp32)
  - ranking: seg top-8 -> 3 value-rounds on 64 candidates -> 3 full-row
    max_index calls (measured ~1.5us each) instead of 3 full-row rounds
  - c-tables stored bf16 (half the gather traffic); rank-0 (self) read as a
    contiguous DMA instead of an indirect gather (19 gathers/tile, not 20)
  - max-aggregation: bf16 packed tensor_tensor tree (L1), tensor_reduce (L0)
  - BN sum-of-squares of gathered rows via G^T G diag-block matmuls on PE
    (kills the Act Square passes over gathered data)
  - exact BN stats (sampling fails tolerance), allreduce as baseline
"""

import numpy as np

import concourse.bacc as bacc
import concourse.bass as bass
import concourse.mybir as mybir
import concourse.tile as tile
from concourse.bass import IndirectOffsetOnAxis
from concourse.bass_utils import run_bass_kernel_spmd

F32 = mybir.dt.float32
F32R = mybir.dt.float32r
BF16 = mybir.dt.bfloat16
U32 = mybir.dt.uint32
AX = mybir.AxisListType
OP = mybir.AluOpType
ACTF = mybir.ActivationFunctionType

B = 4
N = 4096
K = 20
HALF = N // 2
NT = HALF // 128          # 16 row tiles
NSEG = 8
SEG = N // NSEG           # 512
NJC = N // 128            # 32 j-chunks for c tables
EPS = 1e-5
SLOPE = 0.2
NEG = -3.0e38
BNK = float(B * N * K)

D0I, D0 = 3, 64
D1I, D1 = 64, 256
DF, DOUT = 512, 256

N_CORES = 8


def _bn_stats_and_affine(nc, pool, dout, suma, sumasq, sumasp, sums, qtot,
                         g_row, be_row, cc_in, cc_out, tag, cc_fn):
    """Combine per-core partial sums into BN scale/shift rows [1, dout]."""
    pack = pool.tile([1, 2 * dout], F32, tag=f"bnpack{tag}")
    nc.vector.scalar_tensor_tensor(pack[:, 0:dout], suma[:, :], float(K),
                                   sums[:, :], op0=OP.mult, op1=OP.add)
    tmp = pool.tile([1, dout], F32, tag=f"bntmp{tag}")
    nc.vector.scalar_tensor_tensor(tmp[:, :], sumasp[:, :], 2.0, qtot[:, :],
                                   op0=OP.mult, op1=OP.add)
    nc.vector.scalar_tensor_tensor(pack[:, dout:2 * dout], sumasq[:, :],
                                   float(K), tmp[:, :], op0=OP.mult, op1=OP.add)
    nc.sync.dma_start(cc_in[:, :], pack[:, :])
    cc_fn("AllReduce", OP.add, [list(range(N_CORES))], cc_in[:, :], cc_out[:, :])
    red = pool.tile([1, 2 * dout], F32, tag=f"bnred{tag}")
    nc.sync.dma_start(red[:, :], cc_out[:, :])

    mu = pool.tile([1, dout], F32, tag=f"bnmu{tag}")
    nc.vector.tensor_scalar_mul(mu[:, :], red[:, 0:dout], 1.0 / BNK)
    e2 = pool.tile([1, dout], F32, tag=f"bne2{tag}")
    nc.vector.tensor_scalar_mul(e2[:, :], red[:, dout:2 * dout], 1.0 / BNK)
    musq = pool.tile([1, dout], F32, tag=f"bnmusq{tag}")
    nc.vector.tensor_mul(musq[:, :], mu[:, :], mu[:, :])
    var = pool.tile([1, dout], F32, tag=f"bnvar{tag}")
    nc.vector.tensor_sub(var[:, :], e2[:, :], musq[:, :])
    nc.vector.tensor_scalar_add(var[:, :], var[:, :], EPS)
    sd = pool.tile([1, dout], F32, tag=f"bnsd{tag}")
    nc.scalar.activation(sd[:, :], var[:, :], ACTF.Sqrt)
    inv = pool.tile([1, dout], F32, tag=f"bninv{tag}")
    nc.vector.reciprocal(inv[:, :], sd[:, :])
    scale = pool.tile([1, dout], F32, tag=f"bnscale{tag}")
    nc.vector.tensor_mul(scale[:, :], g_row[:, :], inv[:, :])
    shift = pool.tile([1, dout], F32, tag=f"bnshift{tag}")
    nc.vector.tensor_mul(shift[:, :], mu[:, :], scale[:, :])
    nc.vector.tensor_sub(shift[:, :], be_row[:, :], shift[:, :])
    return scale, shift


def build_program(fake_cc=False):
    nc = bacc.Bacc("TRN2", target_bir_lowering=False, debug=False,
                   num_devices=N_CORES)

    # ---------------- I/O ----------------
    # row 0 = ones (bias trick), rows 1..3 = x^T
    xT_all = nc.dram_tensor("xT_all", [D0I + 1, N], F32, kind="ExternalInput")
    xT_own = nc.dram_tensor("xT_own", [D0I + 1, HALF], F32,
                            kind="ExternalInput")
    eye128 = nc.dram_tensor("eye128", [128, 128], F32, kind="ExternalInput")
    ones_col = nc.dram_tensor("ones_col", [128, 1], F32, kind="ExternalInput")
    ones_row = nc.dram_tensor("ones_row", [1, 128], F32, kind="ExternalInput")
    w_t0 = nc.dram_tensor("w_t0", [D0I + 1, D0], F32, kind="ExternalInput")
    w_cb0 = nc.dram_tensor("w_cb0", [D0I + 1, D0], F32, kind="ExternalInput")
    g0_r = nc.dram_tensor("g0_r", [1, D0], F32, kind="ExternalInput")
    be0_r = nc.dram_tensor("be0_r", [1, D0], F32, kind="ExternalInput")
    w_t1 = nc.dram_tensor("w_t1", [D1I + 1, D1], F32, kind="ExternalInput")
    w_cb1 = nc.dram_tensor("w_cb1", [D1I + 1, D1], F32, kind="ExternalInput")
    g1_r = nc.dram_tensor("g1_r", [1, D1], F32, kind="ExternalInput")
    be1_r = nc.dram_tensor("be1_r", [1, D1], F32, kind="ExternalInput")
    wg_d = nc.dram_tensor("wg_d", [D1, 1], F32, kind="ExternalInput")
    bg_d = nc.dram_tensor("bg_d", [1, 1], F32, kind="ExternalInput")
    wf_d = nc.dram_tensor("wf_d", [D1, DF], F32, kind="ExternalInput")
    bf_d = nc.dram_tensor("bf_d", [1, DF], F32, kind="ExternalInput")
    wl_d = nc.dram_tensor("wl_d", [DF, DOUT], F32, kind="ExternalInput")
    bl_rep = nc.dram_tensor("bl_rep", [B, DOUT], F32, kind="ExternalInput")
    onehot = nc.dram_tensor("onehot", [1, B], F32, kind="ExternalInput")

    out_d = nc.dram_tensor("out", [B, DOUT], F32, kind="ExternalOutput")

    # internal DRAM
    c0_dram = nc.dram_tensor("c0_dram", [N, D0], BF16)
    c1_dram = nc.dram_tensor("c1_dram", [N, D1], BF16)
    h1t_in = nc.dram_tensor("h1t_in", [D1I, HALF], F32R)
    h1t_out = nc.dram_tensor("h1t_out", [2 * D1I, HALF], F32R)
    bn0_in = nc.dram_tensor("bn0_in", [1, 2 * D0], F32)
    bn0_out = nc.dram_tensor("bn0_out", [1, 2 * D0], F32)
    bn1_in = nc.dram_tensor("bn1_in", [1, 2 * D1], F32)
    bn1_out = nc.dram_tensor("bn1_out", [1, 2 * D1], F32)
    pool_in = nc.dram_tensor("pool_in", [B, DF + 1], F32)
    pool_out = nc.dram_tensor("pool_out", [B, DF + 1], F32)

    def _cc(kind, op, groups, cin, cout, nc=nc):
        if not fake_cc:
            nc.gpsimd.collective_compute(
                kind, op, replica_groups=groups,
                ins=[cin.opt()], outs=[cout.opt()])
        else:
            nc.sync.dma_start(cout, cin)

    with tile.TileContext(nc) as tc:
        with tc.tile_pool(name="persist", bufs=1) as P:

            eye = P.tile([128, 128], F32)
            nc.sync.dma_start(eye[:, :], eye128[:, :])
            eyeb = P.tile([128, 128], BF16)
            nc.vector.tensor_copy(eyeb[:, :], eye[:, :])
            onec = P.tile([128, 1], F32)
            nc.sync.dma_start(onec[:, :], ones_col[:, :])
            onecr = P.tile([128, 1], F32R)
            nc.scalar.activation(onecr[:, :], onec[:, :], ACTF.Copy)
            oner = P.tile([1, 128], F32)
            nc.sync.dma_start(oner[:, :], ones_row[:, :])

            # ==========================================================
            def edgeconv_layer(lyr, LP, din, dout, buf_own, buf_all,
                               wt_sb, wcb_sb, g_dr, be_dr, c_dram,
                               cc_in, cc_out, d2_fn):
                """One EdgeConv layer. buf_* are F32R sbuf tiles:
                buf_own [din+1, HALF] (ones row + own feats)
                buf_all [din+1, N]    (ones row + all feats); after the c
                table is built, d2_fn() overwrites row 0 with -0.5*|x_j|^2
                so buf_all doubles as the knn-score rhs. The a-table weight
                row 0 is zero, so it is insensitive to row 0.
                Returns z [128, NT, dout] (a + maxagg, pre-BN; aliases the
                a-table tile) and (scale, shift) rows."""
                kk = din + 1

                # ---- c table -> DRAM (bf16) ----
                with tc.tile_pool(name=f"ctab{lyr}", bufs=3) as CP, \
                     tc.tile_pool(name=f"ctabp{lyr}", bufs=2, space="PSUM") as CPP:
                    for jc in range(NJC):
                        ps = CPP.tile([128, dout], F32)
                        nc.tensor.matmul(ps[:, :],
                                         buf_all[:, jc * 128:(jc + 1) * 128],
                                         wcb_sb[:, :])
                        ct = CP.tile([128, dout], BF16)
                        nc.scalar.activation(ct[:, :], ps[:, :], ACTF.Copy)
                        nc.sync.dma_start(c_dram[jc * 128:(jc + 1) * 128, :],
                                          ct[:, :])

                # row 0 of buf_all: ones -> -0.5*d2 (after c table consumed it)
                d2_fn()

                # ---- a table (own rows, fp32) ----
                a_sb = LP.tile([128, NT, dout], F32, tag=f"a{lyr}")
                with tc.tile_pool(name=f"atab{lyr}", bufs=2, space="PSUM") as APP:
                    for t in range(NT):
                        ps = APP.tile([128, dout], F32)
                        nc.tensor.matmul(ps[:, :],
                                         buf_own[:, t * 128:(t + 1) * 128],
                                         wt_sb[:, :])
                        nc.scalar.activation(a_sb[:, t, :], ps[:, :], ACTF.Copy)

                # ---- BN stat accumulators ----
                sums_red = LP.tile([128, dout], F32, tag=f"ssr{lyr}")
                asp_red = LP.tile([128, dout], F32, tag=f"apr{lyr}")
                asq_red = LP.tile([128, dout], F32, tag=f"aqr{lyr}")
                a_red = LP.tile([128, dout], F32, tag=f"ared{lyr}")
                qtot = LP.tile([1, dout], F32, tag=f"qt{lyr}")

                GTGP_cm = tc.tile_pool(name=f"gtg{lyr}", bufs=1, space="PSUM")
                GTGP = GTGP_cm.__enter__()
                gtg_ps = GTGP.tile([128, dout], F32, tag="gtg")
                nblk = dout // 128 if dout >= 128 else 1
                blkw = min(dout, 128)

                # ---- per-tile: scores -> rank -> gather -> aggregate ----
                with tc.tile_pool(name=f"ssb{lyr}", bufs=3) as SP, \
                     tc.tile_pool(name=f"cand{lyr}", bufs=2) as CAND, \
                     tc.tile_pool(name=f"idx{lyr}", bufs=4) as IDP, \
                     tc.tile_pool(name=f"g{lyr}", bufs=4) as GP, \
                     tc.tile_pool(name=f"tree{lyr}", bufs=1) as TRP, \
                     tc.tile_pool(name=f"ssl{lyr}", bufs=2) as SL, \
                     tc.tile_pool(name=f"selps{lyr}", bufs=4, space="PSUM") as SPP, \
                     tc.tile_pool(name=f"aggps{lyr}", bufs=3, space="PSUM") as AGP:
                    def rank_and_gather(t):
                        # scores: s[i, j] = x_i.x_j - 0.5|x_j|^2  (fp32r)
                        s_sb = SP.tile([128, N], F32, tag="s")
                        lhs = buf_own[:, t * 128:(t + 1) * 128]
                        for q in range(8):
                            ps = SPP.tile([128, 512], F32, tag="sq")
                            nc.tensor.matmul(ps[:, :], lhs,
                                             buf_all[:, q * 512:(q + 1) * 512])
                            nc.scalar.activation(s_sb[:, q * 512:(q + 1) * 512],
                                                 ps[:, :], ACTF.Copy)
                        # per-segment top-8 candidates
                        cand = CAND.tile([128, 64], F32, tag="cand")
                        for sg in range(NSEG):
                            nc.vector.max(cand[:, sg * 8:(sg + 1) * 8],
                                          s_sb[:, sg * SEG:(sg + 1) * SEG])
                        # 3 rounds; interleaving max_index between the cand
                        # ops lets the DVE match unit pipeline the 3 full-row
                        # scans (~5us total instead of ~5us each)
                        idxt = IDP.tile([128, 24], U32, tag="idxt")
                        cw = cand
                        for r in range(3):
                            v8 = CAND.tile([128, 8], F32, tag=f"v8_{r}")
                            nc.vector.max(v8[:, :], cw[:, :])
                            nc.vector.max_index(idxt[:, r * 8:(r + 1) * 8],
                                                v8[:, :], s_sb[:, :])
                            if r < 2:
                                cn = CAND.tile([128, 64], F32, tag=f"cand{r+1}")
                                nc.vector.match_replace(cn[:, :], v8[:, :],
                                                        cw[:, :], NEG)
                                cw = cn
                        # gather neighbors' c rows (bf16, one rank per DMA)
                        g_all = GP.tile([128, K, dout], BF16, tag="g")
                        for r in range(K):
                            nc.gpsimd.indirect_dma_start(
                                out=g_all[:, r, :],
                                out_offset=None,
                                in_=c_dram[:, :],
                                in_offset=IndirectOffsetOnAxis(
                                    ap=idxt[:, r:r + 1], axis=0),
                            )
                        return g_all

                    def aggregate(t, g_all):
                        # max aggregation
                        m1 = TRP.tile([128, dout], BF16, tag="m1")
                        if dout == 256:
                            m10 = TRP.tile([128, 10, dout], BF16, tag="m10")
                            nc.vector.tensor_tensor(m10[:, :, :],
                                                    g_all[:, 0:10, :],
                                                    g_all[:, 10:20, :],
                                                    op=OP.max)
                            m5 = TRP.tile([128, 5, dout], BF16, tag="m5")
                            nc.vector.tensor_tensor(m5[:, :, :], m10[:, 0:5, :],
                                                    m10[:, 5:10, :], op=OP.max)
                            m2 = TRP.tile([128, 2, dout], BF16, tag="m2")
                            nc.vector.tensor_tensor(m2[:, :, :], m5[:, 0:2, :],
                                                    m5[:, 2:4, :], op=OP.max)
                            nc.vector.tensor_tensor(m1[:, :], m2[:, 0, :],
                                                    m2[:, 1, :], op=OP.max)
                            nc.vector.tensor_tensor(m1[:, :], m1[:, :],
                                                    m5[:, 4, :], op=OP.max)
                        else:
                            nc.vector.tensor_reduce(
                                m1[:, :],
                                g_all.rearrange("p r c -> p c r"),
                                axis=AX.X, op=OP.max)
                        # sum aggregation (for BN cross term): psum += eye@g_r
                        ps_sum = AGP.tile([128, dout], F32, tag="pssum")
                        for r in range(K):
                            nc.tensor.matmul(ps_sum[:, :], eyeb[:, :],
                                             g_all[:, r, :],
                                             start=(r == 0), stop=(r == K - 1),
                                             skip_group_check=True)
                        # G^T G diag blocks accumulated over all (t, r)
                        for r in range(K):
                            for cb in range(nblk):
                                sl_ = slice(cb * blkw, (cb + 1) * blkw)
                                nc.tensor.matmul(
                                    gtg_ps[0:blkw, sl_],
                                    g_all[:, r, sl_], g_all[:, r, sl_],
                                    start=(t == 0 and r == 0),
                                    stop=(t == NT - 1 and r == K - 1),
                                    skip_group_check=True)
                        # BN accumulations
                        ssl = SL.tile([128, dout], F32, tag="ssl")
                        nc.scalar.activation(ssl[:, :], ps_sum[:, :], ACTF.Copy)
                        prod = SL.tile([128, dout], F32, tag="prod")
                        nc.vector.tensor_mul(prod[:, :], a_sb[:, t, :], ssl[:, :])
                        asqte = SL.tile([128, dout], F32, tag="asqte")
                        nc.scalar.activation(asqte[:, :], a_sb[:, t, :],
                                             ACTF.Square)
                        if t == 0:
                            nc.vector.tensor_copy(sums_red[:, :], ssl[:, :])
                            nc.vector.tensor_copy(asp_red[:, :], prod[:, :])
                            nc.vector.tensor_copy(asq_red[:, :], asqte[:, :])
                            nc.vector.tensor_copy(a_red[:, :], a_sb[:, t, :])
                        else:
                            nc.vector.tensor_add(sums_red[:, :], sums_red[:, :],
                                                 ssl[:, :])
                            nc.vector.tensor_add(asp_red[:, :], asp_red[:, :],
                                                 prod[:, :])
                            nc.vector.tensor_add(asq_red[:, :], asq_red[:, :],
                                                 asqte[:, :])
                            nc.vector.tensor_add(a_red[:, :], a_red[:, :],
                                                 a_sb[:, t, :])
                        # z = a + maxagg, in place (after all raw-a readers)
                        nc.vector.tensor_tensor(a_sb[:, t, :], a_sb[:, t, :],
                                                m1[:, :], op=OP.add)

                    for t in range(NT):
                        aggregate(t, rank_and_gather(t))

                # qtot: diag of G^T G blocks -> [1, dout]
                with tc.tile_pool(name=f"qt{lyr}", bufs=1) as QT, \
                     tc.tile_pool(name=f"qtp{lyr}", bufs=2, space="PSUM") as QTP:
                    gtg_sb = QT.tile([128, dout], F32, tag="gtgsb")
                    nc.scalar.activation(gtg_sb[:, :], gtg_ps[:, :], ACTF.Copy)
                    masked = QT.tile([128, dout], F32, tag="masked")
                    for cb in range(nblk):
                        sl_ = slice(cb * blkw, (cb + 1) * blkw)
                        nc.vector.tensor_mul(masked[0:blkw, sl_],
                                             gtg_sb[0:blkw, sl_],
                                             eye[0:blkw, 0:blkw])
                    if blkw < 128:
                        nc.vector.memset(masked[blkw:128, :], 0.0)
                    ps_q = QTP.tile([1, dout], F32, tag="psq")
                    nc.tensor.matmul(ps_q[:, :], onec[:, :], masked[:, :])
                    nc.scalar.activation(qtot[:, :], ps_q[:, :], ACTF.Copy)
                GTGP_cm.__exit__(None, None, None)

                # ---- column sums of accumulators ----
                PST_cm = tc.tile_pool(name=f"psStat{lyr}", bufs=2, space="PSUM")
                PST = PST_cm.__enter__()

                def colsum(red_tile, tag):
                    ps = PST.tile([1, dout], F32, tag=f"cs{tag}")
                    nc.tensor.matmul(ps[:, :], onec[:, :], red_tile[:, :])
                    row = LP.tile([1, dout], F32, tag=f"row{tag}")
                    nc.scalar.activation(row[:, :], ps[:, :], ACTF.Copy)
                    return row

                suma = colsum(a_red, f"a{lyr}")
                sumasq = colsum(asq_red, f"aq{lyr}")
                sumasp = colsum(asp_red, f"ap{lyr}")
                sums = colsum(sums_red, f"s{lyr}")

                g_row = LP.tile([1, dout], F32, tag=f"g{lyr}")
                nc.sync.dma_start(g_row[:, :], g_dr[:, :])
                be_row = LP.tile([1, dout], F32, tag=f"be{lyr}")
                nc.sync.dma_start(be_row[:, :], be_dr[:, :])
                scale, shift = _bn_stats_and_affine(
                    nc, LP, dout, suma, sumasq, sumasp, sums, qtot,
                    g_row, be_row, cc_in, cc_out, lyr, _cc)
                PST_cm.__exit__(None, None, None)

                return a_sb, scale, shift

            # ==========================================================
            # Layer 0 input prep (fp32r working buffers via Act copies)
            h1t_own = P.tile([D1I, HALF], F32)
            with tc.tile_pool(name="l0", bufs=1) as L0P:
                wt0_sb = L0P.tile([D0I + 1, D0], F32R)
                wcb0_sb = L0P.tile([D0I + 1, D0], F32R)
                tmpw = L0P.tile([D0I + 1, D0], F32, tag="tmpw")
                nc.sync.dma_start(tmpw[:, :], w_t0[:, :])
                nc.scalar.activation(wt0_sb[:, :], tmpw[:, :], ACTF.Copy)
                tmpw2 = L0P.tile([D0I + 1, D0], F32, tag="tmpw2")
                nc.sync.dma_start(tmpw2[:, :], w_cb0[:, :])
                nc.scalar.activation(wcb0_sb[:, :], tmpw2[:, :], ACTF.Copy)

                xo = L0P.tile([D0I + 1, HALF], F32, tag="xo")
                nc.sync.dma_start(xo[:, :], xT_own[:, :])
                xa = L0P.tile([D0I + 1, N], F32, tag="xa")
                nc.sync.dma_start(xa[:, :], xT_all[:, :])

                buf_own0 = L0P.tile([D0I + 1, HALF], F32R)
                nc.scalar.activation(buf_own0[:, :], xo[:, :], ACTF.Copy)
                buf_a0 = L0P.tile([D0I + 1, N], F32R)
                nc.scalar.activation(buf_a0[:, :], xa[:, :], ACTF.Copy)

                def d2_l0():
                    # overwrite row 0 with -0.5*(1 + |x_j|^2); the uniform
                    # -0.5 shift from the ones row doesn't change ranking
                    with tc.tile_pool(name="d2p0", bufs=1) as DP, \
                         tc.tile_pool(name="d2ps0", bufs=2, space="PSUM") as DPP:
                        sq = DP.tile([D0I + 1, N], F32R)
                        nc.scalar.activation(sq[:, :], buf_a0[:, :], ACTF.Square)
                        for ch in range(N // 512):
                            ps = DPP.tile([1, 512], F32)
                            nc.tensor.matmul(ps[:, :], onecr[0:D0I + 1, :],
                                             sq[:, ch * 512:(ch + 1) * 512])
                            nc.scalar.activation(
                                buf_a0[0:1, ch * 512:(ch + 1) * 512],
                                ps[:, :], ACTF.Copy, scale=-0.5)

                z0, scale0, shift0 = edgeconv_layer(
                    0, L0P, D0I, D0, buf_own0, buf_a0,
                    wt0_sb, wcb0_sb, g0_r, be0_r, c0_dram, bn0_in, bn0_out,
                    d2_l0)

                # BN + leaky in transposed domain -> h1t_own [64, HALF]
                with tc.tile_pool(name="bncol0", bufs=1) as BC, \
                     tc.tile_pool(name="bnps0", bufs=2, space="PSUM") as BPP:
                    ps = BPP.tile([D0, 1], F32, tag="sc")
                    nc.tensor.transpose(ps[:, :], scale0[:, :], eye[0:1, 0:1])
                    scol = BC.tile([D0, 1], F32, tag="scol")
                    nc.scalar.activation(scol[:, :], ps[:, :], ACTF.Copy)
                    ps2 = BPP.tile([D0, 1], F32, tag="sh")
                    nc.tensor.transpose(ps2[:, :], shift0[:, :], eye[0:1, 0:1])
                    hcol = BC.tile([D0, 1], F32, tag="hcol")
                    nc.scalar.activation(hcol[:, :], ps2[:, :], ACTF.Copy)
                    with tc.tile_pool(name="trps0", bufs=2, space="PSUM") as TPP:
                        for t in range(NT):
                            pst = TPP.tile([D0, 128], F32)
                            nc.tensor.transpose(pst[:, :], z0[:, t, :], eye[:, :])
                            nc.vector.tensor_scalar(
                                h1t_own[:, t * 128:(t + 1) * 128], pst[:, :],
                                scol[:, 0:1], hcol[:, 0:1],
                                op0=OP.mult, op1=OP.add)
                    nc.vector.scalar_tensor_tensor(
                        h1t_own[:, :], h1t_own[:, :], SLOPE, h1t_own[:, :],
                        op0=OP.mult, op1=OP.max)

            # exchange halves within each cloud (dram f32r, bitcast write)
            nc.sync.dma_start(h1t_in[:, :], h1t_own[:, :].bitcast(F32R))
            if not fake_cc:
                nc.gpsimd.collective_compute(
                    "AllGather", OP.bypass,
                    replica_groups=[[0, 1], [2, 3], [4, 5], [6, 7]],
                    ins=[h1t_in[:, :].opt()], outs=[h1t_out[:, :].opt()],
                )
            else:
                nc.sync.dma_start(h1t_out[0:D1I, :], h1t_in[:, :])
                nc.sync.dma_start(h1t_out[D1I:2 * D1I, :], h1t_in[:, :])

            # Layer 1
            h2t = P.tile([128, 2, NT, 128], F32)
            with tc.tile_pool(name="l1", bufs=1) as L1P:
                wt1_sb = L1P.tile([D1I + 1, D1], F32R)
                wcb1_sb = L1P.tile([D1I + 1, D1], F32R)
                tmpw = L1P.tile([D1I + 1, D1], F32, tag="tmpw1")
                nc.sync.dma_start(tmpw[:, :], w_t1[:, :])
                nc.scalar.activation(wt1_sb[:, :], tmpw[:, :], ACTF.Copy)
                tmpw2 = L1P.tile([D1I + 1, D1], F32, tag="tmpw21")
                nc.sync.dma_start(tmpw2[:, :], w_cb1[:, :])
                nc.scalar.activation(wcb1_sb[:, :], tmpw2[:, :], ACTF.Copy)

                buf_own1 = L1P.tile([D1I + 1, HALF], F32R)
                nc.sync.dma_start(buf_own1[1:D1I + 1, :], h1t_in[:, :])
                nc.vector.memset(buf_own1[0:1, :].bitcast(F32), 1.0)
                buf_a1 = L1P.tile([D1I + 1, N], F32R)
                nc.sync.dma_start(buf_a1[1:D1I + 1, 0:HALF], h1t_out[0:D1I, :])
                nc.sync.dma_start(buf_a1[1:D1I + 1, HALF:N],
                                  h1t_out[D1I:2 * D1I, :])
                nc.vector.memset(buf_a1[0:1, :].bitcast(F32), 1.0)

                def d2_l1():
                    with tc.tile_pool(name="d2p1", bufs=1) as DP, \
                         tc.tile_pool(name="d2ps1", bufs=2, space="PSUM") as DPP:
                        sq = DP.tile([D1I + 1, N], F32R)
                        nc.scalar.activation(sq[:, :], buf_a1[:, :], ACTF.Square)
                        for ch in range(N // 512):
                            ps = DPP.tile([1, 512], F32)
                            nc.tensor.matmul(ps[:, :], onecr[0:D1I + 1, :],
                                             sq[:, ch * 512:(ch + 1) * 512])
                            nc.scalar.activation(
                                buf_a1[0:1, ch * 512:(ch + 1) * 512],
                                ps[:, :], ACTF.Copy, scale=-0.5)

                z1, scale1, shift1 = edgeconv_layer(
                    1, L1P, D1I, D1, buf_own1, buf_a1,
                    wt1_sb, wcb1_sb, g1_r, be1_r, c1_dram, bn1_in, bn1_out,
                    d2_l1)

                # BN + leaky transposed -> h2t [128, 2, NT, 128]
                with tc.tile_pool(name="bncol1", bufs=1) as BC, \
                     tc.tile_pool(name="bnps1", bufs=2, space="PSUM") as BPP:
                    scol = BC.tile([128, 2], F32, tag="scol")
                    hcol = BC.tile([128, 2], F32, tag="hcol")
                    for cb in range(2):
                        ps = BPP.tile([128, 1], F32, tag="sc")
                        nc.tensor.transpose(ps[:, :],
                                            scale1[:, cb * 128:(cb + 1) * 128],
                                            eye[0:1, 0:1])
                        nc.scalar.activation(scol[:, cb:cb + 1], ps[:, :],
                                             ACTF.Copy)
                        ps2 = BPP.tile([128, 1], F32, tag="sh")
                        nc.tensor.transpose(ps2[:, :],
                                            shift1[:, cb * 128:(cb + 1) * 128],
                                            eye[0:1, 0:1])
                        nc.scalar.activation(hcol[:, cb:cb + 1], ps2[:, :],
                                             ACTF.Copy)
                    with tc.tile_pool(name="trps1", bufs=3, space="PSUM") as TPP:
                        for t in range(NT):
                            for cb in range(2):
                                pst = TPP.tile([128, 128], F32)
                                nc.tensor.transpose(
                                    pst[:, :],
                                    z1[:, t, cb * 128:(cb + 1) * 128],
                                    eye[:, :])
                                nc.vector.tensor_scalar(
                                    h2t[:, cb, t, :], pst[:, :],
                                    scol[:, cb:cb + 1], hcol[:, cb:cb + 1],
                                    op0=OP.mult, op1=OP.add)
                                # leaky per slice so pooling can start on
                                # finished tiles instead of waiting for a
                                # monolithic pass
                                nc.vector.scalar_tensor_tensor(
                                    h2t[:, cb, t, :], h2t[:, cb, t, :], SLOPE,
                                    h2t[:, cb, t, :], op0=OP.mult, op1=OP.max)

            # ---------------- attention pooling ----------------
            with tc.tile_pool(name="poolw", bufs=1) as PW, \
                 tc.tile_pool(name="poolsb", bufs=3) as PSB:
                PLOOP_cm = tc.tile_pool(name="poolloop", bufs=1, space="PSUM")
                PACC = PLOOP_cm.__enter__()
                PFF_cm = tc.tile_pool(name="poolff", bufs=2, space="PSUM")
                PFF = PFF_cm.__enter__()
                wg_sb = PW.tile([128, 2, 1], F32, tag="wg")
                nc.sync.dma_start(wg_sb[:, 0, :], wg_d[0:128, :])
                nc.sync.dma_start(wg_sb[:, 1, :], wg_d[128:256, :])
                wf_sb = PW.tile([128, 2, DF], F32, tag="wf")
                nc.sync.dma_start(wf_sb[:, 0, :], wf_d[0:128, :])
                nc.sync.dma_start(wf_sb[:, 1, :], wf_d[128:256, :])
                bg_sb = PW.tile([1, 1], F32, tag="bg")
                nc.sync.dma_start(bg_sb[:, :], bg_d[:, :])
                bf_sb = PW.tile([1, DF], F32, tag="bf")
                nc.sync.dma_start(bf_sb[:, :], bf_d[:, :])
                bf_rep = PW.tile([128, DF], F32, tag="bfrep")
                bg_rep = PW.tile([128, 1], F32, tag="bgrep")
                with tc.tile_pool(name="bcps", bufs=1, space="PSUM") as BCP:
                    ps_bf = BCP.tile([128, DF], F32, tag="bfps")
                    nc.tensor.matmul(ps_bf[:, :], oner[:, :], bf_sb[:, :])
                    nc.scalar.activation(bf_rep[:, :], ps_bf[:, :], ACTF.Copy)
                    ps_bg = BCP.tile([128, 1], F32, tag="bgps")
                    nc.tensor.matmul(ps_bg[:, :], oner[:, :], bg_sb[:, :])
                    nc.scalar.activation(bg_rep[:, :], ps_bg[:, :], ACTF.Copy)

                e_sb = PSB.tile([128, NT], F32, tag="e")
                ps_num = PACC.tile([1, DF], F32, tag="num")
                ps_den = PACC.tile([1, 1], F32, tag="den")
                for t in range(NT):
                    ps_f = PFF.tile([128, DF], F32, tag="f")
                    ps_g = PFF.tile([128, 1], F32, tag="gt")
                    for cb in range(2):
                        nc.tensor.matmul(ps_f[:, :], h2t[:, cb, t, :],
                                         wf_sb[:, cb, :],
                                         start=(cb == 0), stop=(cb == 1))
                        nc.tensor.matmul(ps_g[:, :], h2t[:, cb, t, :],
                                         wg_sb[:, cb, :],
                                         start=(cb == 0), stop=(cb == 1))
                    f_sb = PSB.tile([128, DF], F32, tag="fsb")
                    nc.vector.tensor_tensor(f_sb[:, :], ps_f[:, :],
                                            bf_rep[:, :], op=OP.add)
                    nc.vector.scalar_tensor_tensor(f_sb[:, :], f_sb[:, :], 0.0,
                                                   f_sb[:, :], op0=OP.mult,
                                                   op1=OP.max)
                    gt = PSB.tile([128, 1], F32, tag="gtsb")
                    nc.vector.tensor_scalar(gt[:, :], ps_g[:, :], bg_rep[:, 0:1],
                                            0.0, op0=OP.add, op1=OP.max)
                    nc.scalar.activation(e_sb[:, t:t + 1], gt[:, :], ACTF.Exp)
                    nc.tensor.matmul(ps_num[:, :], e_sb[:, t:t + 1], f_sb[:, :],
                                     start=(t == 0), stop=(t == NT - 1),
                                     skip_group_check=True)
                    nc.tensor.matmul(ps_den[:, :], e_sb[:, t:t + 1],
                                     onec[:, 0:1],
                                     start=(t == 0), stop=(t == NT - 1),
                                     skip_group_check=True)

                numden = PSB.tile([1, DF + 1], F32, tag="numden")
                nc.scalar.activation(numden[:, 0:DF], ps_num[:, :], ACTF.Copy)
                nc.scalar.activation(numden[:, DF:DF + 1], ps_den[:, :],
                                     ACTF.Copy)
                PFF_cm.__exit__(None, None, None)
                PLOOP_cm.__exit__(None, None, None)

                oh_sb = PW.tile([1, B], F32, tag="oh")
                nc.sync.dma_start(oh_sb[:, :], onehot[:, :])
                PTAIL_cm = tc.tile_pool(name="pooltail", bufs=1, space="PSUM")
                PTAIL = PTAIL_cm.__enter__()
                ps_pp = PTAIL.tile([B, DF + 1], F32, tag="pp")
                nc.tensor.matmul(ps_pp[:, 0:512], oh_sb[:, :], numden[:, 0:512])
                nc.tensor.matmul(ps_pp[:, 512:DF + 1], oh_sb[:, :],
                                 numden[:, 512:DF + 1])
                pp_sb = PSB.tile([B, DF + 1], F32, tag="ppsb")
                nc.scalar.activation(pp_sb[:, 0:DF + 1], ps_pp[:, 0:DF + 1],
                                     ACTF.Copy)
                nc.sync.dma_start(pool_in[:, :], pp_sb[:, :])
                _cc("AllReduce", OP.add, [list(range(N_CORES))],
                    pool_in[:, :], pool_out[:, :])
                pall = PSB.tile([B, DF + 1], F32, tag="pall")
                nc.sync.dma_start(pall[:, :], pool_out[:, :])
                recip = PSB.tile([B, 1], F32, tag="recip")
                nc.vector.reciprocal(recip[:, :], pall[:, DF:DF + 1])
                pooled = PSB.tile([B, DF], F32, tag="pooled")
                nc.vector.tensor_scalar_mul(pooled[:, :], pall[:, 0:DF],
                                            recip[:, 0:1])

                wl_sb = PW.tile([128, 4, DOUT], F32, tag="wl")
                for f in range(4):
                    nc.sync.dma_start(wl_sb[:, f, :],
                                      wl_d[f * 128:(f + 1) * 128, :])
                pooledT = PSB.tile([128, 4, B], F32, tag="pooledT")
                with tc.tile_pool(name="ptps", bufs=2, space="PSUM") as PTP:
                    for f in range(4):
                        ps = PTP.tile([128, B], F32)
                        nc.tensor.transpose(ps[:, :],
                                            pooled[:, f * 128:(f + 1) * 128],
                                            eye[0:B, 0:B])
                        nc.scalar.activation(pooledT[:, f, :], ps[:, :],
                                             ACTF.Copy)
                ps_out = PTAIL.tile([B, DOUT], F32, tag="out")
                for f in range(4):
                    nc.tensor.matmul(ps_out[:, :], pooledT[:, f, :],
                                     wl_sb[:, f, :],
                                     start=(f == 0), stop=(f == 3))
                blr = PW.tile([B, DOUT], F32, tag="blr")
                nc.sync.dma_start(blr[:, :], bl_rep[:, :])
                res = PSB.tile([B, DOUT], F32, tag="res")
                nc.vector.tensor_tensor(res[:, :], ps_out[:, :], blr[:, :],
                                        op=OP.add)
                nc.sync.dma_start(out_d[:, :], res[:, :])
                PTAIL_cm.__exit__(None, None, None)

    nc.compile()
    return nc


_NC_CACHE = None


def _get_program():
    global _NC_CACHE
    if _NC_CACHE is None:
        _NC_CACHE = build_program()
    return _NC_CACHE


def make_in_maps(inputs):
    x = np.asarray(inputs["x"], dtype=np.float32)
    Wt0 = np.asarray(inputs["Wt0"], np.float32)
    bt0 = np.asarray(inputs["bt0"], np.float32)
    Wp0 = np.asarray(inputs["Wp0"], np.float32)
    bp0 = np.asarray(inputs["bp0"], np.float32)
    g0 = np.asarray(inputs["g0"], np.float32)
    be0 = np.asarray(inputs["be0"], np.float32)
    Wt1 = np.asarray(inputs["Wt1"], np.float32)
    bt1 = np.asarray(inputs["bt1"], np.float32)
    Wp1 = np.asarray(inputs["Wp1"], np.float32)
    bp1 = np.asarray(inputs["bp1"], np.float32)
    g1 = np.asarray(inputs["g1"], np.float32)
    be1 = np.asarray(inputs["be1"], np.float32)
    Wg = np.asarray(inputs["Wg"], np.float32)
    bg = np.asarray(inputs["bg"], np.float32)
    Wf = np.asarray(inputs["Wf"], np.float32)
    bf = np.asarray(inputs["bf"], np.float32)
    Wl = np.asarray(inputs["Wl"], np.float32)
    bl = np.asarray(inputs["bl"], np.float32)

    w_cb0 = np.concatenate([(bt0 + bp0)[None, :], Wp0 - Wt0], axis=0)
    w_cb1 = np.concatenate([(bt1 + bp1)[None, :], Wp1 - Wt1], axis=0)
    w_t0p = np.concatenate([np.zeros((1, D0), np.float32), Wt0], axis=0)
    w_t1p = np.concatenate([np.zeros((1, D1), np.float32), Wt1], axis=0)

    common = {
        "eye128": np.eye(128, dtype=np.float32),
        "ones_col": np.ones((128, 1), np.float32),
        "ones_row": np.ones((1, 128), np.float32),
        "w_t0": w_t0p, "w_cb0": w_cb0,
        "g0_r": g0[None, :], "be0_r": be0[None, :],
        "w_t1": w_t1p, "w_cb1": w_cb1,
        "g1_r": g1[None, :], "be1_r": be1[None, :],
        "wg_d": Wg, "bg_d": bg[None, :],
        "wf_d": Wf, "bf_d": bf[None, :],
        "wl_d": Wl, "bl_rep": np.broadcast_to(bl[None, :], (B, DOUT)).copy(),
    }

    in_maps = []
    ones_n = np.ones((1, N), np.float32)
    for c in range(N_CORES):
        b, h = c // 2, c % 2
        xT_all = np.concatenate([ones_n, x[b].T], axis=0)
        xT_own = np.concatenate([ones_n[:, :HALF],
                                 x[b, h * HALF:(h + 1) * HALF].T], axis=0)
        xT_all = np.ascontiguousarray(xT_all)
        xT_own = np.ascontiguousarray(xT_own)
        oh = np.zeros((1, B), np.float32)
        oh[0, b] = 1.0
        m = dict(common)
        m["xT_all"] = xT_all
        m["xT_own"] = xT_own
        m["onehot"] = oh
        in_maps.append(m)
    return in_maps


def kernel(**inputs):
    in_maps = make_in_maps(inputs)
    nc = _get_program()
    res = run_bass_kernel_spmd(nc, in_maps, core_ids=list(range(N_CORES)))
    return res.results[0]["out"].astype(np.float32)
